# revision 8
# baseline (speedup 1.0000x reference)
"""Trainium2 Bass kernel: multi-head attention (B=8, N=1024, C=768, H=12).

Sharding: pure data-parallel — batch dim (8) maps 1:1 onto the 8 NeuronCores;
weights are replicated. No collectives.

Per-core algorithm (one batch element, all compute in bf16 w/ fp32 PSUM accum):
  1. qkT = [wq*scale; wk] @ x.T            -> [1536, 1024]  (head-dim on partitions)
  2. V   = x @ wv.T                        -> [1024, 768]   (tokens on partitions)
     stored interleaved with a ones-column per head ("Vaug", [*, 65] per head)
  3. per head h: S.T tiles = kT_h.T-matmul (K=64, two heads row-packed via
     tile_position) -> exp on ScalarE -> P.T (unnormalized, bf16)
  4. AV: out.T_h(+denom) = [V_h | 1].T-matmul-accum over nk   (M=65)
     row 64 = softmax denominator; normalize rows 0..63 via
     reciprocal_approx_fast + partition-broadcast DMA + VectorE multiply
  5. y.T = wp.T-matmul(outT) + bias        -> [768, 1024] fp32 -> DRAM

Host side transposes x / weights into the layouts above (bf16) and transposes
the [768, 1024] per-core outputs back into [8, 1024, 768] fp32.
"""
import sys

sys.path.insert(0, "/opt/trn_rl_repo")

import numpy as np
import ml_dtypes

import concourse.bass as bass  # noqa: F401  (registers AP helpers)
import concourse.mybir as mybir
import concourse.tile as tile
from concourse import bacc
from concourse.bass_utils import run_bass_kernel_spmd

B, N, C, H, HD = 8, 1024, 768, 12, 64
KC = C // 128          # 6   contraction chunks of 128 over C
FT = (2 * C) // 128    # 12  q+k feature tiles of 128
NT = N // 128          # 8   token tiles of 128
NQ = N // 512          # 2   query chunks of 512
G = H // 2             # 6   head pairs
BF16 = mybir.dt.bfloat16
F32 = mybir.dt.float32
EXP = mybir.ActivationFunctionType.Exp

_CACHE = {}


def _build(dbg=False):
    nc = bacc.Bacc("TRN2", target_bir_lowering=False, debug=False,
                   enable_asserts=False, num_devices=B)
    xt_d = nc.dram_tensor("xt", [C, N], BF16, kind="ExternalInput").ap()
    wqk_d = nc.dram_tensor("wqk", [C, 2 * C], BF16, kind="ExternalInput").ap()
    wv_d = nc.dram_tensor("wv", [C, C], BF16, kind="ExternalInput").ap()
    wp_d = nc.dram_tensor("wp", [C, C], BF16, kind="ExternalInput").ap()
    bp_d = nc.dram_tensor("bp", [128, KC], F32, kind="ExternalInput").ap()
    out_d = nc.dram_tensor("out", [C, N], F32, kind="ExternalOutput").ap()
    if dbg:
        dbg_qk = nc.dram_tensor("dbg_qk", [128, FT * N], BF16, kind="ExternalOutput").ap()
        dbg_v = nc.dram_tensor("dbg_v", [128, NT * H * 65], BF16, kind="ExternalOutput").ap()
        dbg_pt = nc.dram_tensor("dbg_pt", [128, H * NT * 1024], BF16, kind="ExternalOutput").ap()
        dbg_outT = nc.dram_tensor("dbg_outT", [128, KC * N], BF16, kind="ExternalOutput").ap()
        dbg_av = nc.dram_tensor("dbg_av", [65, H * 2 * 512], F32, kind="ExternalOutput").ap()
        dbg_rec = nc.dram_tensor("dbg_rec", [1, H * 2 * 512], F32, kind="ExternalOutput").ap()
        dbg_rb = nc.dram_tensor("dbg_rb", [64, H * 2 * 512], F32, kind="ExternalOutput").ap()

    with tile.TileContext(nc) as tc:
        with (
            tc.tile_pool(name="wpool", bufs=1) as wpool,
            tc.tile_pool(name="big", bufs=1) as big,
            tc.tile_pool(name="pt", bufs=20) as ptpool,
            tc.tile_pool(name="small", bufs=6) as small,
            tc.tile_pool(name="ye", bufs=3) as yepool,
            tc.tile_pool(name="psA", bufs=2, space="PSUM") as psA,
            tc.tile_pool(name="psS", bufs=2, space="PSUM") as psS,
            tc.tile_pool(name="psV", bufs=2, space="PSUM") as psV,
        ):
            xt = wpool.tile([128, KC * N], BF16)        # x.T  chunks
            wqk = wpool.tile([128, KC * 2 * C], BF16)
            wv = wpool.tile([128, KC * C], BF16)
            wp = wpool.tile([128, KC * C], BF16)
            bp = wpool.tile([128, KC], F32)
            qk = big.tile([128, FT * N], BF16)          # qkT feature tiles
            vsb = big.tile([128, NT * H * 65], BF16)    # Vaug: (nt, h) -> 65 cols
            outT = big.tile([128, KC * N], BF16)        # attn out, transposed

            for k in range(KC):
                nc.sync.dma_start(out=xt[:, k * N:(k + 1) * N],
                                  in_=xt_d[k * 128:(k + 1) * 128, :])
                nc.sync.dma_start(out=wqk[:, k * 2 * C:(k + 1) * 2 * C],
                                  in_=wqk_d[k * 128:(k + 1) * 128, :])
                nc.sync.dma_start(out=wv[:, k * C:(k + 1) * C],
                                  in_=wv_d[k * 128:(k + 1) * 128, :])
                nc.sync.dma_start(out=wp[:, k * C:(k + 1) * C],
                                  in_=wp_d[k * 128:(k + 1) * 128, :])
            nc.sync.dma_start(out=bp[:], in_=bp_d)

            v3 = vsb[:].rearrange("p (a b) -> p a b", b=65)  # a = nt*H + h
            nc.vector.memset(v3[:, :, 64:65], 1.0)           # ones columns

            def qkT_tiles(t):
                for nq in range(NQ):
                    ps = psA.tile([128, 512], F32, tag="psA")
                    for k in range(KC):
                        nc.tensor.matmul(
                            ps[:],
                            lhsT=wqk[:, k * 2 * C + t * 128: k * 2 * C + (t + 1) * 128],
                            rhs=xt[:, k * N + nq * 512: k * N + nq * 512 + 512],
                            start=(k == 0), stop=(k == KC - 1))
                    nc.vector.tensor_copy(
                        out=qk[:, t * N + nq * 512: t * N + nq * 512 + 512],
                        in_=ps[:])

            def v_tiles(nt):
                for fc in range(2):
                    ps = psA.tile([128, 384], F32, tag="psA")
                    for k in range(KC):
                        nc.tensor.matmul(
                            ps[:],
                            lhsT=xt[:, k * N + nt * 128: k * N + nt * 128 + 128],
                            rhs=wv[:, k * C + fc * 384: k * C + fc * 384 + 384],
                            start=(k == 0), stop=(k == KC - 1))
                    dst = v3[:, nt * H + fc * 6: nt * H + fc * 6 + 6, 0:64]
                    nc.vector.tensor_copy(out=dst,
                                          in_=ps[:].rearrange("p (a b) -> p a b", b=64))

            pts = {}

            def scores(g):
                for nk in range(NT):
                    for half in range(2):
                        h = 2 * g + half
                        ps = psS.tile([128, 2 * 512], F32, tag="psS")
                        for nq in range(NQ):
                            nc.tensor.matmul(
                                ps[:, nq * 512:(nq + 1) * 512],
                                lhsT=qk[half * 64:(half + 1) * 64,
                                        (6 + g) * N + nk * 128: (6 + g) * N + nk * 128 + 128],
                                rhs=qk[half * 64:(half + 1) * 64,
                                       g * N + nq * 512: g * N + nq * 512 + 512],
                                start=True, stop=True)
                        pt = ptpool.tile([128, 2 * 512], BF16, tag="pt")
                        nc.scalar.activation(out=pt[:], in_=ps[:], func=EXP)
                        pts[(h, nk)] = pt
                        if dbg:
                            off = (h * NT + nk) * 1024
                            nc.sync.dma_start(out=dbg_pt[:, off:off + 1024], in_=pt[:])

            def av(h):
                g, half = divmod(h, 2)
                for nq in range(NQ):
                    ps = psV.tile([128, 512], F32, tag="psV")
                    for nk in range(NT):
                        nc.tensor.matmul(
                            ps[0:65, :],
                            lhsT=v3[:, nk * H + h, :],
                            rhs=pts[(h, nk)][:, nq * 512:(nq + 1) * 512],
                            start=(nk == 0), stop=(nk == NT - 1))
                    st65 = small.tile([65, 512], F32, tag="st65")
                    nc.vector.tensor_copy(out=st65[:], in_=ps[0:65, :])
                    rb = small.tile([64, 512], F32, tag="rb")
                    nc.sync.dma_start(
                        out=rb[:],
                        in_=st65[64:65, :][:, None, :].broadcast_to([1, 64, 512]))
                    rec64 = small.tile([64, 512], F32, tag="rec64")
                    nc.vector.reciprocal_approx_fast(out=rec64[:], in_=rb[:])
                    if dbg:
                        off = (h * 2 + nq) * 512
                        nc.sync.dma_start(out=dbg_av[:, off:off + 512], in_=st65[:])
                        nc.sync.dma_start(out=dbg_rec[:, off:off + 512], in_=rec64[0:1, :])
                        nc.sync.dma_start(out=dbg_rb[:, off:off + 512], in_=rb[:])
                    dst = outT[half * 64:(half + 1) * 64,
                               g * N + nq * 512: g * N + nq * 512 + 512]
                    if half == 0:
                        nc.vector.tensor_mul(out=dst, in0=st65[0:64, :], in1=rec64[:])
                    else:
                        st = small.tile([64, 512], BF16, tag="st")
                        nc.vector.tensor_mul(out=st[:], in0=st65[0:64, :], in1=rec64[:])
                        nc.sync.dma_start(out=dst, in_=st[:])

            def proj():
                for t in range(KC):
                    for nq in range(NQ):
                        ps = psA.tile([128, 512], F32, tag="psA")
                        for k in range(KC):
                            nc.tensor.matmul(
                                ps[:],
                                lhsT=wp[:, k * C + t * 128: k * C + (t + 1) * 128],
                                rhs=outT[:, k * N + nq * 512: k * N + nq * 512 + 512],
                                start=(k == 0), stop=(k == KC - 1))
                        ye = yepool.tile([128, 512], F32, tag="ye")
                        nc.vector.tensor_scalar_add(out=ye[:], in0=ps[:],
                                                    scalar1=bp[:, t:t + 1])
                        nc.sync.dma_start(
                            out=out_d[t * 128:(t + 1) * 128, nq * 512: nq * 512 + 512],
                            in_=ye[:])

            # emission order == scheduling priority
            qkT_tiles(0)
            qkT_tiles(6)
            for nt in range(NT):
                v_tiles(nt)
            scores(0)
            qkT_tiles(1)
            qkT_tiles(7)
            for g in range(G):
                av(2 * g)
                av(2 * g + 1)
                if g + 1 < G:
                    scores(g + 1)
                if g + 2 < G:
                    qkT_tiles(g + 2)
                    qkT_tiles(6 + g + 2)
            if dbg:
                nc.sync.dma_start(out=dbg_qk[:], in_=qk[:])
                nc.sync.dma_start(out=dbg_v[:], in_=vsb[:])
                nc.sync.dma_start(out=dbg_outT[:], in_=outT[:])
            proj()

    nc.compile()
    return nc


def _get_nc():
    if "nc" not in _CACHE:
        _CACHE["nc"] = _build()
    return _CACHE["nc"]


def _prep_inputs(x, w_qkv, w_proj, b_proj):
    bf16 = ml_dtypes.bfloat16
    scale = np.float32(HD) ** -0.5
    wqk = np.concatenate([w_qkv[:C] * scale, w_qkv[C:2 * C]], axis=0)
    wqkT = np.ascontiguousarray(wqk.T).astype(bf16)
    wvT = np.ascontiguousarray(w_qkv[2 * C:].T).astype(bf16)
    wpT = np.ascontiguousarray(w_proj.T).astype(bf16)
    bpT = np.ascontiguousarray(b_proj.astype(np.float32).reshape(KC, 128).T)
    in_maps = []
    for c in range(B):
        xT = np.ascontiguousarray(x[c].T).astype(bf16)
        in_maps.append({"xt": xT, "wqk": wqkT, "wv": wvT, "wp": wpT, "bp": bpT})
    return in_maps


def run(inputs, trace=False):
    nc = _get_nc()
    in_maps = _prep_inputs(**inputs)
    res = run_bass_kernel_spmd(nc, in_maps, core_ids=list(range(B)), trace=trace)
    out = np.stack([np.asarray(res.results[c]["out"]).T for c in range(B)], axis=0)
    return np.ascontiguousarray(out.astype(np.float32)), res


def kernel(x, w_qkv, w_proj, b_proj):
    out, _ = run(dict(x=np.asarray(x), w_qkv=np.asarray(w_qkv),
                      w_proj=np.asarray(w_proj), b_proj=np.asarray(b_proj)))
    return out


# revision 13
# speedup vs baseline: 1.0019x; 1.0019x over previous
"""Trainium2 Bass kernel: multi-head attention (B=8, N=1024, C=768, H=12).

Sharding: pure data-parallel — batch dim (8) maps 1:1 onto the 8 NeuronCores;
weights are replicated. No collectives.

Per-core algorithm (one batch element, all compute in bf16 w/ fp32 PSUM accum):
  1. qkT = [wq*scale; wk] @ x.T            -> [1536, 1024]  (head-dim on partitions)
  2. V   = x @ wv.T                        -> [1024, 768]   (tokens on partitions)
     stored interleaved with a ones-column per head ("Vaug", [*, 65] per head)
  3. per head h: S.T tiles = kT_h.T-matmul (K=64, two heads row-packed via
     tile_position) -> exp on ScalarE -> P.T (unnormalized, bf16)
  4. AV: out.T_h(+denom) = [V_h | 1].T-matmul-accum over nk   (M=65)
     row 64 = softmax denominator; normalize rows 0..63 via
     reciprocal_approx_fast + partition-broadcast DMA + VectorE multiply
  5. y.T = wp.T-matmul(outT) + bias        -> [768, 1024] fp32 -> DRAM

Host side transposes x / weights into the layouts above (bf16) and transposes
the [768, 1024] per-core outputs back into [8, 1024, 768] fp32.
"""
import sys

sys.path.insert(0, "/opt/trn_rl_repo")

import numpy as np
import ml_dtypes

import concourse.bass as bass  # noqa: F401  (registers AP helpers)
import concourse.mybir as mybir
import concourse.tile as tile
from concourse import bacc
from concourse.bass_utils import run_bass_kernel_spmd

B, N, C, H, HD = 8, 1024, 768, 12, 64
KC = C // 128          # 6   contraction chunks of 128 over C
FT = (2 * C) // 128    # 12  q+k feature tiles of 128
NT = N // 128          # 8   token tiles of 128
NQ = N // 512          # 2   query chunks of 512
G = H // 2             # 6   head pairs
BF16 = mybir.dt.bfloat16
F32 = mybir.dt.float32
EXP = mybir.ActivationFunctionType.Exp

_CACHE = {}


def _build(dbg=False):
    nc = bacc.Bacc("TRN2", target_bir_lowering=False, debug=False,
                   enable_asserts=False, num_devices=B)
    xt_d = nc.dram_tensor("xt", [C, N], BF16, kind="ExternalInput").ap()
    wqk_d = nc.dram_tensor("wqk", [C, 2 * C], BF16, kind="ExternalInput").ap()
    wv_d = nc.dram_tensor("wv", [C, C], BF16, kind="ExternalInput").ap()
    wp_d = nc.dram_tensor("wp", [C, C], BF16, kind="ExternalInput").ap()
    bp_d = nc.dram_tensor("bp", [128, KC], F32, kind="ExternalInput").ap()
    out_d = nc.dram_tensor("out", [C, N], F32, kind="ExternalOutput").ap()
    if dbg:
        dbg_qk = nc.dram_tensor("dbg_qk", [128, FT * N], BF16, kind="ExternalOutput").ap()
        dbg_v = nc.dram_tensor("dbg_v", [128, NT * H * 65], BF16, kind="ExternalOutput").ap()
        dbg_pt = nc.dram_tensor("dbg_pt", [128, H * NT * 1024], BF16, kind="ExternalOutput").ap()
        dbg_outT = nc.dram_tensor("dbg_outT", [128, KC * N], BF16, kind="ExternalOutput").ap()
        dbg_av = nc.dram_tensor("dbg_av", [65, H * 2 * 512], F32, kind="ExternalOutput").ap()
        dbg_rec = nc.dram_tensor("dbg_rec", [1, H * 2 * 512], F32, kind="ExternalOutput").ap()
        dbg_rb = nc.dram_tensor("dbg_rb", [64, H * 2 * 512], F32, kind="ExternalOutput").ap()

    with tile.TileContext(nc) as tc:
        with (
            tc.tile_pool(name="wpool", bufs=1) as wpool,
            tc.tile_pool(name="big", bufs=1) as big,
            tc.tile_pool(name="pt", bufs=20) as ptpool,
            tc.tile_pool(name="small", bufs=6) as small,
            tc.tile_pool(name="ye", bufs=3) as yepool,
            tc.tile_pool(name="psA", bufs=2, space="PSUM") as psA,
            tc.tile_pool(name="psS", bufs=2, space="PSUM") as psS,
            tc.tile_pool(name="psV", bufs=2, space="PSUM") as psV,
        ):
            xt = wpool.tile([128, KC * N], BF16)        # x.T  chunks
            wqk = wpool.tile([128, KC * 2 * C], BF16)
            wv = wpool.tile([128, KC * C], BF16)
            wp = wpool.tile([128, KC * C], BF16)
            bp = wpool.tile([128, KC], F32)
            qk = big.tile([128, FT * N], BF16)          # qkT feature tiles
            vsb = big.tile([128, NT * H * 65], BF16)    # Vaug: (nt, h) -> 65 cols
            outTs = [big.tile([128, N], BF16, tag=f"outT{g}", name=f"outT{g}")
                     for g in range(G)]

            # spread input loads over independent DMA queues
            for k in range(KC):
                nc.sync.dma_start(out=xt[:, k * N:(k + 1) * N],
                                  in_=xt_d[k * 128:(k + 1) * 128, :])
                nc.gpsimd.dma_start(out=wqk[:, k * 2 * C:(k + 1) * 2 * C],
                                    in_=wqk_d[k * 128:(k + 1) * 128, :])
                nc.scalar.dma_start(out=wv[:, k * C:(k + 1) * C],
                                    in_=wv_d[k * 128:(k + 1) * 128, :])
                nc.gpsimd.dma_start(out=wp[:, k * C:(k + 1) * C],
                                    in_=wp_d[k * 128:(k + 1) * 128, :])
            nc.scalar.dma_start(out=bp[:], in_=bp_d)

            v3 = vsb[:].rearrange("p (a b) -> p a b", b=65)  # a = nt*H + h
            nc.vector.memset(v3[:, :, 64:65], 1.0)           # ones columns

            def qkT_tiles(t):
                for nq in range(NQ):
                    ps = psA.tile([128, 512], F32, tag="psA")
                    for k in range(KC):
                        nc.tensor.matmul(
                            ps[:],
                            lhsT=wqk[:, k * 2 * C + t * 128: k * 2 * C + (t + 1) * 128],
                            rhs=xt[:, k * N + nq * 512: k * N + nq * 512 + 512],
                            start=(k == 0), stop=(k == KC - 1))
                    nc.vector.tensor_copy(
                        out=qk[:, t * N + nq * 512: t * N + nq * 512 + 512],
                        in_=ps[:])

            def v_tiles(nt):
                for fc in range(2):
                    ps = psA.tile([128, 384], F32, tag="psA")
                    for k in range(KC):
                        nc.tensor.matmul(
                            ps[:],
                            lhsT=xt[:, k * N + nt * 128: k * N + nt * 128 + 128],
                            rhs=wv[:, k * C + fc * 384: k * C + fc * 384 + 384],
                            start=(k == 0), stop=(k == KC - 1))
                    dst = v3[:, nt * H + fc * 6: nt * H + fc * 6 + 6, 0:64]
                    nc.vector.tensor_copy(out=dst,
                                          in_=ps[:].rearrange("p (a b) -> p a b", b=64))

            pts = {}

            def scores(g):
                for nk in range(NT):
                    for half in range(2):
                        h = 2 * g + half
                        ps = psS.tile([128, 2 * 512], F32, tag="psS")
                        for nq in range(NQ):
                            nc.tensor.matmul(
                                ps[:, nq * 512:(nq + 1) * 512],
                                lhsT=qk[half * 64:(half + 1) * 64,
                                        (6 + g) * N + nk * 128: (6 + g) * N + nk * 128 + 128],
                                rhs=qk[half * 64:(half + 1) * 64,
                                       g * N + nq * 512: g * N + nq * 512 + 512],
                                start=True, stop=True)
                        pt = ptpool.tile([128, 2 * 512], BF16, tag="pt")
                        nc.scalar.activation(out=pt[:], in_=ps[:], func=EXP)
                        pts[(h, nk)] = pt
                        if dbg:
                            off = (h * NT + nk) * 1024
                            nc.sync.dma_start(out=dbg_pt[:, off:off + 1024], in_=pt[:])

            def av(h):
                g, half = divmod(h, 2)
                for nq in range(NQ):
                    ps = psV.tile([128, 512], F32, tag="psV")
                    for nk in range(NT):
                        nc.tensor.matmul(
                            ps[0:65, :],
                            lhsT=v3[:, nk * H + h, :],
                            rhs=pts[(h, nk)][:, nq * 512:(nq + 1) * 512],
                            start=(nk == 0), stop=(nk == NT - 1))
                    st65 = small.tile([65, 512], F32, tag="st65")
                    nc.vector.tensor_copy(out=st65[:], in_=ps[0:65, :])
                    rb = small.tile([64, 512], F32, tag="rb")
                    nc.sync.dma_start(
                        out=rb[:],
                        in_=st65[64:65, :][:, None, :].broadcast_to([1, 64, 512]))
                    rec64 = small.tile([64, 512], F32, tag="rec64")
                    nc.vector.reciprocal_approx_fast(out=rec64[:], in_=rb[:])
                    if dbg:
                        off = (h * 2 + nq) * 512
                        nc.sync.dma_start(out=dbg_av[:, off:off + 512], in_=st65[:])
                        nc.sync.dma_start(out=dbg_rec[:, off:off + 512], in_=rec64[0:1, :])
                        nc.sync.dma_start(out=dbg_rb[:, off:off + 512], in_=rb[:])
                    dst = outTs[g][half * 64:(half + 1) * 64,
                                   nq * 512: nq * 512 + 512]
                    if half == 0:
                        nc.vector.tensor_mul(out=dst, in0=st65[0:64, :], in1=rec64[:])
                    else:
                        st = small.tile([64, 512], BF16, tag="st")
                        nc.vector.tensor_mul(out=st[:], in0=st65[0:64, :], in1=rec64[:])
                        nc.gpsimd.dma_start(out=dst, in_=st[:])

            def proj():
                for t in range(KC):
                    for nq in range(NQ):
                        ps = psA.tile([128, 512], F32, tag="psA")
                        for k in range(KC):
                            nc.tensor.matmul(
                                ps[:],
                                lhsT=wp[:, k * C + t * 128: k * C + (t + 1) * 128],
                                rhs=outTs[k][:, nq * 512: nq * 512 + 512],
                                start=(k == 0), stop=(k == KC - 1))
                        ye = yepool.tile([128, 512], F32, tag="ye")
                        nc.vector.tensor_scalar_add(out=ye[:], in0=ps[:],
                                                    scalar1=bp[:, t:t + 1])
                        nc.sync.dma_start(
                            out=out_d[t * 128:(t + 1) * 128, nq * 512: nq * 512 + 512],
                            in_=ye[:])

            # emission order == scheduling priority: scores(0) as early as
            # possible so ScalarE (the pace-setting engine) starts exp'ing;
            # qkT pair g+1 emitted inside iteration g as PE filler.
            qkT_tiles(0)
            qkT_tiles(6)
            scores(0)
            for nt in range(NT):
                v_tiles(nt)
            for g in range(G):
                av(2 * g)
                av(2 * g + 1)
                if g + 1 < G:
                    qkT_tiles(g + 1)
                    qkT_tiles(6 + g + 1)
                    scores(g + 1)
            if dbg:
                nc.sync.dma_start(out=dbg_qk[:], in_=qk[:])
                nc.sync.dma_start(out=dbg_v[:], in_=vsb[:])
                for g in range(G):
                    nc.sync.dma_start(out=dbg_outT[:, g * N:(g + 1) * N], in_=outTs[g][:])
            proj()

    nc.compile()
    return nc


def _get_nc():
    if "nc" not in _CACHE:
        _CACHE["nc"] = _build()
    return _CACHE["nc"]


def _prep_inputs(x, w_qkv, w_proj, b_proj):
    bf16 = ml_dtypes.bfloat16
    scale = np.float32(HD) ** -0.5
    wqk = np.concatenate([w_qkv[:C] * scale, w_qkv[C:2 * C]], axis=0)
    wqkT = np.ascontiguousarray(wqk.T).astype(bf16)
    wvT = np.ascontiguousarray(w_qkv[2 * C:].T).astype(bf16)
    wpT = np.ascontiguousarray(w_proj.T).astype(bf16)
    bpT = np.ascontiguousarray(b_proj.astype(np.float32).reshape(KC, 128).T)
    in_maps = []
    for c in range(B):
        xT = np.ascontiguousarray(x[c].T).astype(bf16)
        in_maps.append({"xt": xT, "wqk": wqkT, "wv": wvT, "wp": wpT, "bp": bpT})
    return in_maps


def run(inputs, trace=False):
    nc = _get_nc()
    in_maps = _prep_inputs(**inputs)
    res = run_bass_kernel_spmd(nc, in_maps, core_ids=list(range(B)), trace=trace)
    out = np.stack([np.asarray(res.results[c]["out"]).T for c in range(B)], axis=0)
    return np.ascontiguousarray(out.astype(np.float32)), res


def kernel(x, w_qkv, w_proj, b_proj):
    out, _ = run(dict(x=np.asarray(x), w_qkv=np.asarray(w_qkv),
                      w_proj=np.asarray(w_proj), b_proj=np.asarray(b_proj)))
    return out
